# revision 4
# baseline (speedup 1.0000x reference)
"""GNN message-passing (R-GCN style) kernel for 8 Trainium2 NeuronCores.

Reference computation:
    msgs = einsum("eoi,ei->eo", W[widx], x[u])      # per-edge transform
    out  = relu(segment_sum(msgs, v, N))            # scatter-add + relu

Distribution strategy: edges are sharded by destination-node range
(12500 nodes per core), so each core owns a disjoint slice of the output
and no inter-core collective is needed.  W and x are replicated.

Device-side work (all FLOPs):
  Launch A: per-edge weight transform.  Edges are grouped by weight index
    (widx); each group's edges are packed 8-per-column and processed with
    a block-diagonal [128,128] @ [128,ncols] PE matmul (K = 8x16).  The
    block-diagonal weight operand is built on-device in SBUF from a
    compact j-replicated weight bank (1MB instead of shipping the 8.4MB
    expanded operand): memset (split across vector/scalar/gpsimd) plus 8
    affine strided DMAs place the 16x16 blocks on the diagonals.  Group
    column ranges are variable (sized to the actual per-group edge
    counts, maxed across cores so one SPMD program serves all 8 cores)
    instead of padded to a fixed 1024 slots.
  Launch B: segment-sum + ReLU.  Destination nodes are bucketed into
    128-node windows by descending degree, so each window is padded only
    to its own max degree (DN_k) instead of a global worst case; windows
    with equal DN_k are batched into single vector tensor_reduce
    instructions.  ReLU on the scalar engine.

The host does data layout only: sharding, sorting/padding into the
static group structure, gathering x rows into the packed matmul operand,
and permuting the 16-float messages from widx-order to v-order between
the two launches.  (Device-side per-edge random access is not available:
the loadable GPSIMD ucode libraries are absent and indirect DMA has
32B/row descriptor granularity, far too slow for 200K rows/core.)
"""

import sys

sys.path.insert(0, "/opt/trn_rl_repo")

import numpy as np
import ml_dtypes

try:
    # bass_utils imports antenv.axon_hooks when tracing is requested via
    # env; some images lack that module — register a graceful stub so a
    # BASS_TRACE=1 environment degrades to "no trace" instead of crashing.
    import antenv.axon_hooks  # noqa: F401
except ImportError:
    import types

    import antenv

    _hooks = types.ModuleType("antenv.axon_hooks")
    _hooks._hook = None
    _hooks.set_axon_ntff_profile_hook = lambda h: setattr(_hooks, "_hook", h)
    _hooks.get_axon_ntff_profile_hook = lambda: _hooks._hook
    sys.modules["antenv.axon_hooks"] = _hooks
    antenv.axon_hooks = _hooks

import concourse.bacc as bacc
import concourse.mybir as mybir
import concourse.tile as tile
from concourse.bass_utils import run_bass_kernel_spmd

BF16 = ml_dtypes.bfloat16

# set by test harnesses: when True, launches run with trace=True and
# per-launch exec times land in LAST_EXEC_NS
TRACE = False
LAST_EXEC_NS = []

N_NODES = 100000
D = 16
NW = 256
N_CORES = 8
VSH = N_NODES // N_CORES          # 12500 destination nodes per core

CHUNK = 2048                      # A-side columns per DMA chunk
NWIN = (VSH + 127) // 128         # 98 destination 128-node windows per core
B_MAX_FREE = 8192                 # B-side max elems/partition per sbuf tile


def _build_kernel_a(TCP, units_by_chunk):
    """units_by_chunk[ch] = list of (g, c0, c1) absolute column ranges."""
    nc = bacc.Bacc(None, target_bir_lowering=False, debug=False)
    XU = nc.dram_tensor("XU", [128, TCP], mybir.dt.bfloat16, kind="ExternalInput")
    W8 = nc.dram_tensor("W8", [128, NW * D], mybir.dt.bfloat16, kind="ExternalInput")
    MSG = nc.dram_tensor("MSG", [128, TCP], mybir.dt.bfloat16, kind="ExternalOutput")

    with tile.TileContext(nc) as tc:
        with (
            tc.tile_pool(name="bd", bufs=1) as bdp,
            tc.tile_pool(name="sbuf", bufs=3) as pool,
            tc.tile_pool(name="psum", bufs=2, space="PSUM") as psum_pool,
        ):
            # ---- build block-diagonal weight operand in SBUF -----------
            bd = bdp.tile([128, NW * 128], mybir.dt.bfloat16, tag="bd")
            # zero-fill split across three engines (disjoint ranges)
            nc.vector.memset(bd[:, 0:18432], 0)
            nc.scalar.memzero(bd[:, 18432:28672])
            nc.gpsimd.memset(bd[:, 28672:NW * 128], 0)
            # 8 strided DMAs place W[g,o,i] on the j-th diagonal block:
            # bd[16j+i, 128g+16j+o] = W8[16j+i, 16g+o] = W[g,o,i]
            for j in range(8):
                nc.sync.dma_start(
                    out=bd[16 * j:16 * (j + 1), :]
                    .rearrange("p (g c) -> p g c", c=128)[:, :, 16 * j:16 * (j + 1)],
                    in_=W8[16 * j:16 * (j + 1), :]
                    .rearrange("p (g o) -> p g o", o=D),
                )

            nchunks = TCP // CHUNK
            for ch in range(nchunks):
                base = ch * CHUNK
                xu_t = pool.tile([128, CHUNK], mybir.dt.bfloat16, tag="xu")
                nc.sync.dma_start(out=xu_t[:], in_=XU[:, base:base + CHUNK])
                out_t = pool.tile([128, CHUNK], mybir.dt.bfloat16, tag="out")
                for i, (g, c0, c1) in enumerate(units_by_chunk[ch]):
                    n = c1 - c0
                    ps = psum_pool.tile([128, 128], mybir.dt.float32, tag=f"ps{i % 4}")
                    nc.tensor.matmul(
                        out=ps[:, :n],
                        lhsT=bd[:, 128 * g:128 * (g + 1)],
                        rhs=xu_t[:, c0 - base:c1 - base],
                        start=True,
                        stop=True,
                    )
                    nc.scalar.copy(out=out_t[:, c0 - base:c1 - base], in_=ps[:, :n])
                nc.sync.dma_start(out=MSG[:, base:base + CHUNK], in_=out_t[:])
    nc.compile()
    return nc


def _build_kernel_b(TOTB, runs):
    """runs = list of (dn, k0, k1, woff) equal-DN window runs (chunked)."""
    nc = bacc.Bacc(None, target_bir_lowering=False, debug=False)
    MSGB = nc.dram_tensor("MSGB", [TOTB], mybir.dt.bfloat16, kind="ExternalInput")
    OUTP = nc.dram_tensor("OUTP", [NWIN * 128, D], mybir.dt.float32, kind="ExternalOutput")

    with tile.TileContext(nc) as tc:
        with tc.tile_pool(name="sbuf", bufs=4) as pool:
            for dn, k0, k1, woff in runs:
                nw = k1 - k0
                msg_t = pool.tile([128, nw * D * dn], mybir.dt.bfloat16, tag="msg")
                nc.sync.dma_start(
                    out=msg_t[:].rearrange("p (w o s) -> p w o s", w=nw, o=D),
                    in_=MSGB[woff:woff + nw * 128 * D * dn]
                    .rearrange("(w p o s) -> p w o s", w=nw, p=128, o=D),
                )
                acc_t = pool.tile([128, nw * D], mybir.dt.float32, tag="acc")
                nc.vector.tensor_reduce(
                    out=acc_t[:],
                    in_=msg_t[:].rearrange("p (w o s) -> p w o s", w=nw, o=D),
                    axis=mybir.AxisListType.X,
                    op=mybir.AluOpType.add,
                )
                out_t = pool.tile([128, nw * D], mybir.dt.float32, tag="out")
                nc.scalar.activation(out_t[:], acc_t[:], mybir.ActivationFunctionType.Relu)
                nc.sync.dma_start(
                    out=OUTP[k0 * 128:k1 * 128, :].rearrange("(w p) o -> p w o", w=nw),
                    in_=out_t[:].rearrange("p (w o) -> p w o", w=nw),
                )
    nc.compile()
    return nc


def _prep_a(u_s, widx_s, x_bf, NC, colofs, TCP):
    """Pack one core's gathered x rows into the A-side matmul operand.

    Returns (XU [128, TCP] bf16, col(edge), j(edge)) where edge order is
    the stable widx sort of this core's edges.
    """
    ordA = np.argsort(widx_s, kind="stable")
    wA = widx_s[ordA]
    n = u_s.shape[0]
    cnts = np.bincount(wA, minlength=NW)
    starts = np.zeros(NW + 1, np.int64)
    np.cumsum(cnts, out=starts[1:])
    rank = np.arange(n) - starts[wA]
    col = colofs[wA] + rank // 8
    j = rank % 8

    xu3 = np.zeros((TCP * 8, D), BF16)
    xu3[col * 8 + j] = x_bf[u_s[ordA]]
    # [TCP, 8, 16] -> [8, 16, TCP] -> [128, TCP], row = 16j+i
    XU = np.ascontiguousarray(
        xu3.reshape(TCP, 8, D).transpose(1, 2, 0).reshape(128, TCP)
    )
    col_of_edge = np.empty(n, np.int64)
    col_of_edge[ordA] = col
    j_of_edge = np.empty(n, np.int64)
    j_of_edge[ordA] = j
    return XU, col_of_edge, j_of_edge


def kernel(x, W, u, v, widx):
    x = np.asarray(x, np.float32)
    W = np.asarray(W, np.float32)
    u = np.asarray(u).astype(np.int64)
    v = np.asarray(v).astype(np.int64)
    widx = np.asarray(widx).astype(np.int64)

    x_bf = x.astype(BF16)

    # compact j-replicated weight bank: W8[16j+i, 16g+o] = W[g, o, i]
    WT = W.transpose(0, 2, 1)                          # [g, i, o]
    W8 = np.broadcast_to(WT.transpose(1, 0, 2)[None], (8, D, NW, D))
    W8 = np.ascontiguousarray(W8.reshape(128, NW * D)).astype(BF16)

    # ---- shard by destination range -----------------------------------
    shard = v // VSH
    sel = [shard == m for m in range(N_CORES)]
    u_s = [u[s] for s in sel]
    v_s = [v[s] - m * VSH for m, s in enumerate(sel)]
    w_s = [widx[s] for s in sel]

    # ---- common A-side structure (max group size across cores) --------
    cnts = np.stack([np.bincount(ws, minlength=NW) for ws in w_s])   # [8, NW]
    NC = (cnts.max(axis=0) + 7) // 8                                 # cols per group
    NC = np.maximum(NC, 1)
    colofs = np.zeros(NW + 1, np.int64)
    np.cumsum(NC, out=colofs[1:])
    TC = int(colofs[-1])
    TCP = ((TC + CHUNK - 1) // CHUNK) * CHUNK
    nchunks = TCP // CHUNK

    units_by_chunk = [[] for _ in range(nchunks)]
    for g in range(NW):
        c = int(colofs[g])
        b = c + int(NC[g])
        while c < b:
            ch = c // CHUNK
            lim = min(b, (ch + 1) * CHUNK, c + 128)
            units_by_chunk[ch].append((g, c, lim))
            c = lim

    # ---- common B-side structure (degree-sorted windows) --------------
    degs = np.stack([np.bincount(vs, minlength=VSH) for vs in v_s])  # [8, VSH]
    perms = [np.argsort(-degs[m], kind="stable") for m in range(N_CORES)]
    sdeg = np.stack([degs[m][perms[m]] for m in range(N_CORES)])     # desc
    DN = sdeg[:, ::128].max(axis=0).astype(np.int64)                 # [NWIN]
    DN = np.maximum(DN, 1)
    woff = np.zeros(NWIN + 1, np.int64)
    np.cumsum(DN * 128 * D, out=woff[1:])
    TOTB = int(woff[-1])

    runs = []
    k = 0
    while k < NWIN:
        k2 = k
        while k2 < NWIN and DN[k2] == DN[k]:
            k2 += 1
        # chunk runs so each sbuf tile stays small
        dn = int(DN[k])
        max_nw = max(1, B_MAX_FREE // (D * dn))
        while k < k2:
            k1 = min(k2, k + max_nw)
            runs.append((dn, k, k1, int(woff[k])))
            k = k1

    # ---- host prep per core -------------------------------------------
    prepsA = [_prep_a(u_s[m], w_s[m], x_bf, NC, colofs, TCP) for m in range(N_CORES)]

    # ---- launch A: per-edge transform ---------------------------------
    ncA = _build_kernel_a(TCP, units_by_chunk)
    in_maps_a = [{"XU": p[0], "W8": W8} for p in prepsA]
    LAST_EXEC_NS.clear()
    resA = run_bass_kernel_spmd(ncA, in_maps_a, list(range(N_CORES)), trace=TRACE)
    if TRACE:
        LAST_EXEC_NS.append(resA.exec_time_ns)

    # ---- host: permute messages widx-order -> v-order -----------------
    in_maps_b = []
    for m in range(N_CORES):
        msgsA = resA.results[m]["MSG"]                # [128, TCP] bf16
        _, col, j = prepsA[m]
        vecs = msgsA[(j * D)[:, None] + np.arange(D)[None, :], col[:, None]]

        vs = v_s[m]
        ordB = np.argsort(vs, kind="stable")
        vB = vs[ordB]
        deg = degs[m]
        startsB = np.zeros(VSH + 1, np.int64)
        np.cumsum(deg, out=startsB[1:])
        s_of = np.arange(vB.shape[0]) - startsB[vB]   # slot within node
        rank_of_node = np.empty(VSH, np.int64)
        rank_of_node[perms[m]] = np.arange(VSH)
        r = rank_of_node[vB]
        kw = r // 128
        p = r % 128
        dnk = DN[kw]
        base = woff[kw] + (p * D) * dnk + s_of
        flat = np.zeros(TOTB, BF16)
        flat[base[:, None] + np.arange(D)[None, :] * dnk[:, None]] = vecs[ordB]
        in_maps_b.append({"MSGB": flat})

    # ---- launch B: segment-sum + ReLU ---------------------------------
    ncB = _build_kernel_b(TOTB, runs)
    resB = run_bass_kernel_spmd(ncB, in_maps_b, list(range(N_CORES)), trace=TRACE)
    if TRACE:
        LAST_EXEC_NS.append(resB.exec_time_ns)

    out = np.empty((N_NODES, D), np.float32)
    for m in range(N_CORES):
        outP = resB.results[m]["OUTP"]                # [NWIN*128, D] f32
        out[m * VSH + perms[m]] = outP[:VSH]
    return out


# revision 9
# speedup vs baseline: 1.2221x; 1.2221x over previous
"""GNN message-passing (R-GCN style) kernel for 8 Trainium2 NeuronCores.

Reference computation:
    msgs = einsum("eoi,ei->eo", W[widx], x[u])      # per-edge transform
    out  = relu(segment_sum(msgs, v, N))            # scatter-add + relu

Distribution strategy: edges are sharded by destination-node range
(12500 nodes per core), so each core owns a disjoint slice of the output
and no inter-core collective is needed.  W and x are replicated.

Device-side work (all FLOPs):
  Launch A: per-edge weight transform.  Edges are grouped by weight index
    (widx); each group's edges are packed 8-per-column and processed with
    a block-diagonal [128,128] @ [128,ncols] PE matmul (K = 8x16).  The
    block-diagonal weight operand lives in SBUF striped by j-slot
    (stripe j holds the weight bank contiguously at segment j), so it is
    built from the compact 1MB j-replicated bank W8 with 8 fully
    contiguous DMAs plus a zero memset split across three engines; the
    matmul lhsT reads it with a strided [128, 8, 16] access pattern.
    Group column ranges are variable (sized to actual per-group counts,
    maxed across cores so one SPMD program serves all 8 cores).  PSUM is
    accumulated in [128, 512] blocks drained by single large copies
    alternating between the vector and scalar engines.
  Launch B: segment-sum + ReLU.  Destination nodes are bucketed into
    128-node windows by descending degree, so each window is padded only
    to its own max degree (DN_k); windows with equal DN_k are batched
    into single vector tensor_reduce instructions (bf16 in/out engages
    the DVE 2x mode; sums of <=44 bf16 values keep absmax error well
    inside the 2e-2 budget).  ReLU on the scalar engine.

The host does data layout only: sharding, sorting/padding into the
static group structure, gathering x rows into the packed matmul operand,
and permuting the 16-float messages from widx-order to v-order between
the two launches.  (Device-side per-edge random access is not available:
the loadable GPSIMD ucode libraries are absent and indirect DMA has
32B/row descriptor granularity, far too slow for 200K rows/core.)
"""

import sys

sys.path.insert(0, "/opt/trn_rl_repo")

import numpy as np
import ml_dtypes

try:
    # bass_utils imports antenv.axon_hooks when tracing is requested via
    # env; some images lack that module — register a graceful stub so a
    # BASS_TRACE=1 environment degrades to "no trace" instead of crashing.
    import antenv.axon_hooks  # noqa: F401
except ImportError:
    import types

    import antenv

    _hooks = types.ModuleType("antenv.axon_hooks")
    _hooks._hook = None
    _hooks.set_axon_ntff_profile_hook = lambda h: setattr(_hooks, "_hook", h)
    _hooks.get_axon_ntff_profile_hook = lambda: _hooks._hook
    sys.modules["antenv.axon_hooks"] = _hooks
    antenv.axon_hooks = _hooks

import concourse.bacc as bacc
import concourse.mybir as mybir
import concourse.tile as tile
from concourse.bass_utils import run_bass_kernel_spmd

BF16 = ml_dtypes.bfloat16

# set by test harnesses: when True, launches run with trace=True and
# per-launch exec times land in LAST_EXEC_NS
TRACE = False
LAST_EXEC_NS = []

N_NODES = 100000
D = 16
NW = 256
N_CORES = 8
VSH = N_NODES // N_CORES          # 12500 destination nodes per core

CHUNK = 2048                      # A-side columns per DMA chunk
PSB = 512                         # A-side columns per PSUM block
NWIN = (VSH + 127) // 128         # 98 destination 128-node windows per core
B_MAX_FREE = 4096                 # B-side max elems/partition per sbuf tile
B_OUT_BF16 = True                 # B-side reduce/relu/output in bf16 (DVE 2x)


def _build_kernel_a(TCP, units):
    """units = list of (g, c0, c1) absolute column ranges, ascending, each
    within a single PSB-aligned block (and hence a single CHUNK)."""
    nc = bacc.Bacc(None, target_bir_lowering=False, debug=False)
    XU = nc.dram_tensor("XU", [128, TCP], mybir.dt.bfloat16, kind="ExternalInput")
    W8 = nc.dram_tensor("W8", [128, NW * D], mybir.dt.bfloat16, kind="ExternalInput")
    MSG = nc.dram_tensor("MSG", [128, TCP], mybir.dt.bfloat16, kind="ExternalOutput")

    # group units by psum block
    blocks = {}
    for g, c0, c1 in units:
        blocks.setdefault(c0 // PSB, []).append((g, c0, c1))

    with tile.TileContext(nc) as tc:
        with (
            tc.tile_pool(name="bd", bufs=1) as bdp,
            tc.tile_pool(name="sbuf", bufs=3) as pool,
            tc.tile_pool(name="psum", bufs=2, space="PSUM") as psum_pool,
        ):
            # ---- build striped block-diagonal weight operand in SBUF ---
            # stripe j: partitions [16j,16j+16) hold the full bank at
            # segment j (cols [4096j, 4096j+4096)), zeros elsewhere.
            bd = bdp.tile([128, 8 * NW * D], mybir.dt.bfloat16, tag="bd")
            nc.vector.memset(bd[:, 0:14336], 0)
            nc.scalar.memzero(bd[:, 14336:24576])
            nc.gpsimd.memset(bd[:, 24576:8 * NW * D], 0)
            for j in range(8):
                nc.sync.dma_start(
                    out=bd[16 * j:16 * (j + 1), 4096 * j:4096 * (j + 1)],
                    in_=W8[16 * j:16 * (j + 1), :],
                )
            bdv = bd[:].rearrange("p (m g) -> p m g", g=NW)

            nchunks = TCP // CHUNK
            ncopy = 0
            for ch in range(nchunks):
                base = ch * CHUNK
                xu_t = pool.tile([128, CHUNK], mybir.dt.bfloat16, tag="xu")
                nc.sync.dma_start(out=xu_t[:], in_=XU[:, base:base + CHUNK])
                out_t = pool.tile([128, CHUNK], mybir.dt.bfloat16, tag="out")
                for b in range(base // PSB, (base + CHUNK) // PSB):
                    us = blocks.get(b, [])
                    if not us:
                        continue
                    ps = psum_pool.tile([128, PSB], mybir.dt.float32, tag=f"ps{b % 4}")
                    for g, c0, c1 in us:
                        nc.tensor.matmul(
                            out=ps[:, c0 - b * PSB:c1 - b * PSB],
                            lhsT=bdv[:, :, g:g + 1],
                            rhs=xu_t[:, c0 - base:c1 - base],
                            start=True,
                            stop=True,
                        )
                    lo_ps = us[0][1] - b * PSB
                    hi_ps = us[-1][2] - b * PSB
                    lo = us[0][1] - base
                    hi = us[-1][2] - base
                    if ncopy % 2 == 0:
                        nc.vector.tensor_copy(out_t[:, lo:hi], ps[:, lo_ps:hi_ps])
                    else:
                        nc.scalar.copy(out=out_t[:, lo:hi], in_=ps[:, lo_ps:hi_ps])
                    ncopy += 1
                nc.sync.dma_start(out=MSG[:, base:base + CHUNK], in_=out_t[:])
    nc.compile()
    return nc


def _build_kernel_b(TOTB, runs):
    """runs = list of (dn, k0, k1, woff) equal-DN window runs (chunked)."""
    nc = bacc.Bacc(None, target_bir_lowering=False, debug=False)
    odt = mybir.dt.bfloat16 if B_OUT_BF16 else mybir.dt.float32
    MSGB = nc.dram_tensor("MSGB", [TOTB], mybir.dt.bfloat16, kind="ExternalInput")
    OUTP = nc.dram_tensor("OUTP", [NWIN * 128, D], odt, kind="ExternalOutput")

    with nc.allow_low_precision("node sums of <=44 bf16 terms fit error budget"), \
            tile.TileContext(nc) as tc:
        with tc.tile_pool(name="sbuf", bufs=6) as pool:
            for dn, k0, k1, woff in runs:
                nw = k1 - k0
                msg_t = pool.tile([128, nw * D * dn], mybir.dt.bfloat16, tag="msg")
                nc.sync.dma_start(
                    out=msg_t[:].rearrange("p (w o s) -> p w o s", w=nw, o=D),
                    in_=MSGB[woff:woff + nw * 128 * D * dn]
                    .rearrange("(w p o s) -> p w o s", w=nw, p=128, o=D),
                )
                acc_t = pool.tile([128, nw * D], odt, tag="acc")
                nc.vector.tensor_reduce(
                    out=acc_t[:],
                    in_=msg_t[:].rearrange("p (w o s) -> p w o s", w=nw, o=D),
                    axis=mybir.AxisListType.X,
                    op=mybir.AluOpType.add,
                )
                out_t = pool.tile([128, nw * D], odt, tag="out")
                nc.scalar.activation(out_t[:], acc_t[:], mybir.ActivationFunctionType.Relu)
                nc.sync.dma_start(
                    out=OUTP[k0 * 128:k1 * 128, :].rearrange("(w p) o -> p w o", w=nw),
                    in_=out_t[:].rearrange("p (w o) -> p w o", w=nw),
                )
    nc.compile()
    return nc


def _prep_a(u_s, widx_s, x_bf, colofs, TCP):
    """Pack one core's gathered x rows into the A-side matmul operand.

    Returns (XU [128, TCP] bf16, col(edge), j(edge)) where edge order is
    the stable widx sort of this core's edges.
    """
    ordA = np.argsort(widx_s, kind="stable")
    wA = widx_s[ordA]
    n = u_s.shape[0]
    cnts = np.bincount(wA, minlength=NW)
    starts = np.zeros(NW + 1, np.int64)
    np.cumsum(cnts, out=starts[1:])
    rank = np.arange(n) - starts[wA]
    col = colofs[wA] + rank // 8
    j = rank % 8

    xu3 = np.zeros((TCP * 8, D), BF16)
    xu3[col * 8 + j] = x_bf[u_s[ordA]]
    # [TCP, 8, 16] -> [8, 16, TCP] -> [128, TCP], row = 16j+i
    XU = np.ascontiguousarray(
        xu3.reshape(TCP, 8, D).transpose(1, 2, 0).reshape(128, TCP)
    )
    col_of_edge = np.empty(n, np.int64)
    col_of_edge[ordA] = col
    j_of_edge = np.empty(n, np.int64)
    j_of_edge[ordA] = j
    return XU, col_of_edge, j_of_edge


def kernel(x, W, u, v, widx):
    x = np.asarray(x, np.float32)
    W = np.asarray(W, np.float32)
    u = np.asarray(u).astype(np.int64)
    v = np.asarray(v).astype(np.int64)
    widx = np.asarray(widx).astype(np.int64)

    x_bf = x.astype(BF16)

    # compact j-replicated weight bank: W8[16j+i, 256o+g] = W[g, o, i]
    # (bank-transposed so a group's lhsT is a single-stride AP: the
    # striped SBUF operand bd[16j+i, 256*(16j+o)+g] reads as stride-256)
    WT = W.transpose(2, 1, 0)                          # [i, o, g]
    W8 = np.broadcast_to(WT[None], (8, D, D, NW))
    W8 = np.ascontiguousarray(W8.reshape(128, NW * D)).astype(BF16)

    # ---- shard by destination range -----------------------------------
    shard = v // VSH
    sel = [shard == m for m in range(N_CORES)]
    u_s = [u[s] for s in sel]
    v_s = [v[s] - m * VSH for m, s in enumerate(sel)]
    w_s = [widx[s] for s in sel]

    # ---- common A-side structure (max group size across cores) --------
    cnts = np.stack([np.bincount(ws, minlength=NW) for ws in w_s])   # [8, NW]
    NC = (cnts.max(axis=0) + 7) // 8                                 # cols per group
    NC = np.maximum(NC, 1)
    colofs = np.zeros(NW + 1, np.int64)
    np.cumsum(NC, out=colofs[1:])
    TC = int(colofs[-1])
    TCP = ((TC + CHUNK - 1) // CHUNK) * CHUNK

    units = []
    for g in range(NW):
        c = int(colofs[g])
        b = c + int(NC[g])
        while c < b:
            lim = min(b, (c // PSB + 1) * PSB)
            units.append((g, c, lim))
            c = lim

    # ---- common B-side structure (degree-sorted windows) --------------
    degs = np.stack([np.bincount(vs, minlength=VSH) for vs in v_s])  # [8, VSH]
    perms = [np.argsort(-degs[m], kind="stable") for m in range(N_CORES)]
    sdeg = np.stack([degs[m][perms[m]] for m in range(N_CORES)])     # desc
    DN = sdeg[:, ::128].max(axis=0).astype(np.int64)                 # [NWIN]
    DN = np.maximum(DN, 1)
    woff = np.zeros(NWIN + 1, np.int64)
    np.cumsum(DN * 128 * D, out=woff[1:])
    TOTB = int(woff[-1])

    runs = []
    k = 0
    while k < NWIN:
        k2 = k
        while k2 < NWIN and DN[k2] == DN[k]:
            k2 += 1
        # chunk runs so each sbuf tile stays small
        dn = int(DN[k])
        max_nw = max(1, B_MAX_FREE // (D * dn))
        while k < k2:
            k1 = min(k2, k + max_nw)
            runs.append((dn, k, k1, int(woff[k])))
            k = k1

    # ---- host prep per core -------------------------------------------
    prepsA = [_prep_a(u_s[m], w_s[m], x_bf, colofs, TCP) for m in range(N_CORES)]

    # ---- launch A: per-edge transform ---------------------------------
    ncA = _build_kernel_a(TCP, units)
    in_maps_a = [{"XU": p[0], "W8": W8} for p in prepsA]
    LAST_EXEC_NS.clear()
    resA = run_bass_kernel_spmd(ncA, in_maps_a, list(range(N_CORES)), trace=TRACE)
    if TRACE:
        LAST_EXEC_NS.append(resA.exec_time_ns)

    # ---- host: permute messages widx-order -> v-order -----------------
    in_maps_b = []
    for m in range(N_CORES):
        msgsA = resA.results[m]["MSG"]                # [128, TCP] bf16
        _, col, j = prepsA[m]
        vecs = msgsA[(j * D)[:, None] + np.arange(D)[None, :], col[:, None]]

        vs = v_s[m]
        ordB = np.argsort(vs, kind="stable")
        vB = vs[ordB]
        deg = degs[m]
        startsB = np.zeros(VSH + 1, np.int64)
        np.cumsum(deg, out=startsB[1:])
        s_of = np.arange(vB.shape[0]) - startsB[vB]   # slot within node
        rank_of_node = np.empty(VSH, np.int64)
        rank_of_node[perms[m]] = np.arange(VSH)
        r = rank_of_node[vB]
        kw = r // 128
        p = r % 128
        dnk = DN[kw]
        base = woff[kw] + (p * D) * dnk + s_of
        flat = np.zeros(TOTB, BF16)
        flat[base[:, None] + np.arange(D)[None, :] * dnk[:, None]] = vecs[ordB]
        in_maps_b.append({"MSGB": flat})

    # ---- launch B: segment-sum + ReLU ---------------------------------
    ncB = _build_kernel_b(TOTB, runs)
    resB = run_bass_kernel_spmd(ncB, in_maps_b, list(range(N_CORES)), trace=TRACE)
    if TRACE:
        LAST_EXEC_NS.append(resB.exec_time_ns)

    out = np.empty((N_NODES, D), np.float32)
    for m in range(N_CORES):
        outP = resB.results[m]["OUTP"]                # [NWIN*128, D]
        out[m * VSH + perms[m]] = outP[:VSH].astype(np.float32)
    return out


# revision 13
# speedup vs baseline: 1.3155x; 1.0764x over previous
"""GNN message-passing (R-GCN style) kernel for 8 Trainium2 NeuronCores.

Reference computation:
    msgs = einsum("eoi,ei->eo", W[widx], x[u])      # per-edge transform
    out  = relu(segment_sum(msgs, v, N))            # scatter-add + relu

Distribution strategy: edges are sharded by destination-node range
(12500 nodes per core), so each core owns a disjoint slice of the output
and no inter-core collective is needed.  W and x are replicated.

Device-side work (all FLOPs):
  Launch A: per-edge weight transform.  Edges are grouped by weight index
    (widx); each group's edges are packed 8-per-column and processed with
    a block-diagonal [128,128] @ [128,ncols] PE matmul (K = 8x16).  The
    block-diagonal weight operand lives in SBUF striped by j-slot
    (stripe j holds the weight bank contiguously at segment j), so it is
    built from the compact 1MB j-replicated bank W8 with 8 fully
    contiguous DMAs plus a zero memset split across three engines; the
    matmul lhsT reads it with a strided [128, 8, 16] access pattern.
    Group column ranges are variable (sized to actual per-group counts,
    maxed across cores so one SPMD program serves all 8 cores).  PSUM is
    accumulated in [128, 512] blocks drained by single large copies
    alternating between the vector and scalar engines.
  Launch B: segment-sum + ReLU.  Destination nodes are bucketed into
    128-node windows by descending degree, so each window is padded only
    to its own max degree (DN_k); windows with equal DN_k are batched
    into single vector tensor_reduce instructions (bf16 in/out engages
    the DVE 2x mode; sums of <=44 bf16 values keep absmax error well
    inside the 2e-2 budget).  ReLU on the scalar engine.

The host does data layout only: sharding, sorting/padding into the
static group structure, gathering x rows into the packed matmul operand,
and permuting the 16-float messages from widx-order to v-order between
the two launches.  (Device-side per-edge random access is not available:
the loadable GPSIMD ucode libraries are absent and indirect DMA has
32B/row descriptor granularity, far too slow for 200K rows/core.)
"""

import sys

sys.path.insert(0, "/opt/trn_rl_repo")

import numpy as np
import ml_dtypes

try:
    # bass_utils imports antenv.axon_hooks when tracing is requested via
    # env; some images lack that module — register a graceful stub so a
    # BASS_TRACE=1 environment degrades to "no trace" instead of crashing.
    import antenv.axon_hooks  # noqa: F401
except ImportError:
    import types

    import antenv

    _hooks = types.ModuleType("antenv.axon_hooks")
    _hooks._hook = None
    _hooks.set_axon_ntff_profile_hook = lambda h: setattr(_hooks, "_hook", h)
    _hooks.get_axon_ntff_profile_hook = lambda: _hooks._hook
    sys.modules["antenv.axon_hooks"] = _hooks
    antenv.axon_hooks = _hooks

import concourse.bacc as bacc
import concourse.mybir as mybir
import concourse.tile as tile
from concourse.bass_utils import run_bass_kernel_spmd

BF16 = ml_dtypes.bfloat16

# set by test harnesses: when True, launches run with trace=True and
# per-launch exec times land in LAST_EXEC_NS
TRACE = False
LAST_EXEC_NS = []

N_NODES = 100000
D = 16
NW = 256
N_CORES = 8
VSH = N_NODES // N_CORES          # 12500 destination nodes per core

CHUNK = 2048                      # A-side columns per DMA chunk
PSB = 512                         # A-side columns per PSUM block
NWIN = (VSH + 127) // 128         # 98 destination 128-node windows per core
B_MAX_FREE = 4096                 # B-side max elems/partition per sbuf tile
B_OUT_BF16 = True                 # B-side reduce/relu/output in bf16 (DVE 2x)


def _build_kernel_a(TCP, units):
    """units = list of (g, c0, c1) absolute column ranges, ascending, each
    within a single PSB-aligned block (and hence a single CHUNK)."""
    nc = bacc.Bacc(None, target_bir_lowering=False, debug=False)
    XU = nc.dram_tensor("XU", [128, TCP], mybir.dt.bfloat16, kind="ExternalInput")
    W8 = nc.dram_tensor("W8", [128, NW * D], mybir.dt.bfloat16, kind="ExternalInput")
    MSG = nc.dram_tensor("MSG", [128, TCP], mybir.dt.bfloat16, kind="ExternalOutput")

    # group units by psum block
    blocks = {}
    for g, c0, c1 in units:
        blocks.setdefault(c0 // PSB, []).append((g, c0, c1))

    with tile.TileContext(nc) as tc:
        with (
            tc.tile_pool(name="bd", bufs=1) as bdp,
            tc.tile_pool(name="sbuf", bufs=3) as pool,
            tc.tile_pool(name="psum", bufs=2, space="PSUM") as psum_pool,
        ):
            # ---- build striped block-diagonal weight operand in SBUF ---
            # stripe j: partitions [16j,16j+16) hold the full bank at
            # segment j (cols [4096j, 4096j+4096)), zeros elsewhere.
            bd = bdp.tile([128, 8 * NW * D], mybir.dt.bfloat16, tag="bd")
            # memset split across engines (4096-aligned so each stripe
            # build-DMA depends on exactly one memset)
            nc.vector.memset(bd[:, 0:12288], 0)
            nc.scalar.memzero(bd[:, 12288:24576])
            nc.gpsimd.memset(bd[:, 24576:8 * NW * D], 0)
            # stripe builds on the gpsimd queue, own-region stripes first
            for j in (6, 7, 0, 1, 2, 3, 4, 5):
                nc.gpsimd.dma_start(
                    out=bd[16 * j:16 * (j + 1), 4096 * j:4096 * (j + 1)],
                    in_=W8[16 * j:16 * (j + 1), :],
                )
            bdv = bd[:].rearrange("p (m g) -> p m g", g=NW)

            queues = [nc.sync, nc.scalar, nc.gpsimd]
            nchunks = TCP // CHUNK
            ncopy = 0
            for ch in range(nchunks):
                base = ch * CHUNK
                xu_t = pool.tile([128, CHUNK], mybir.dt.bfloat16, tag="xu")
                queues[ch % 3].dma_start(out=xu_t[:], in_=XU[:, base:base + CHUNK])
                out_t = pool.tile([128, CHUNK], mybir.dt.bfloat16, tag="out")
                for b in range(base // PSB, (base + CHUNK) // PSB):
                    us = blocks.get(b, [])
                    if not us:
                        continue
                    ps = psum_pool.tile([128, PSB], mybir.dt.float32, tag=f"ps{b % 4}")
                    for g, c0, c1 in us:
                        nc.tensor.matmul(
                            out=ps[:, c0 - b * PSB:c1 - b * PSB],
                            lhsT=bdv[:, :, g:g + 1],
                            rhs=xu_t[:, c0 - base:c1 - base],
                            start=True,
                            stop=True,
                        )
                    lo_ps = us[0][1] - b * PSB
                    hi_ps = us[-1][2] - b * PSB
                    lo = us[0][1] - base
                    hi = us[-1][2] - base
                    if ncopy % 2 == 0:
                        nc.vector.tensor_copy(out_t[:, lo:hi], ps[:, lo_ps:hi_ps])
                    else:
                        nc.scalar.copy(out=out_t[:, lo:hi], in_=ps[:, lo_ps:hi_ps])
                    ncopy += 1
                queues[(ch + 1) % 3].dma_start(out=MSG[:, base:base + CHUNK], in_=out_t[:])
    nc.compile()
    return nc


def _build_kernel_b(TOTB, runs):
    """runs = list of (dn, k0, k1, woff) equal-DN window runs (chunked)."""
    nc = bacc.Bacc(None, target_bir_lowering=False, debug=False)
    odt = mybir.dt.bfloat16 if B_OUT_BF16 else mybir.dt.float32
    MSGB = nc.dram_tensor("MSGB", [TOTB], mybir.dt.bfloat16, kind="ExternalInput")
    # partition-major output: OUTP[p, k*D+o] = out of node at rank k*128+p
    OUTP = nc.dram_tensor("OUTP", [128, NWIN * D], odt, kind="ExternalOutput")

    with nc.allow_low_precision("node sums of <=44 bf16 terms fit error budget"), \
            tile.TileContext(nc) as tc:
        with tc.tile_pool(name="sbuf", bufs=6) as pool:
            queues = [nc.sync, nc.gpsimd, nc.scalar]
            for ri, (dn, k0, k1, woff) in enumerate(runs):
                nw = k1 - k0
                msg_t = pool.tile([128, nw * D * dn], mybir.dt.bfloat16, tag="msg")
                queues[ri % 3].dma_start(
                    out=msg_t[:].rearrange("p (w o s) -> p w o s", w=nw, o=D),
                    in_=MSGB[woff:woff + nw * 128 * D * dn]
                    .rearrange("(w p o s) -> p w o s", w=nw, p=128, o=D),
                )
                acc_t = pool.tile([128, nw * D], odt, tag="acc")
                nc.vector.tensor_reduce(
                    out=acc_t[:],
                    in_=msg_t[:].rearrange("p (w o s) -> p w o s", w=nw, o=D),
                    axis=mybir.AxisListType.X,
                    op=mybir.AluOpType.add,
                )
                out_t = pool.tile([128, nw * D], odt, tag="out")
                nc.scalar.activation(out_t[:], acc_t[:], mybir.ActivationFunctionType.Relu)
                queues[(ri + 1) % 3].dma_start(
                    out=OUTP[:, k0 * D:k1 * D],
                    in_=out_t[:],
                )
    nc.compile()
    return nc


def _prep_a(u_s, widx_s, x_bf, colofs, TCP):
    """Pack one core's gathered x rows into the A-side matmul operand.

    Returns (XU [128, TCP] bf16, col(edge), j(edge)) where edge order is
    the stable widx sort of this core's edges.
    """
    ordA = np.argsort(widx_s, kind="stable")
    wA = widx_s[ordA]
    n = u_s.shape[0]
    cnts = np.bincount(wA, minlength=NW)
    starts = np.zeros(NW + 1, np.int64)
    np.cumsum(cnts, out=starts[1:])
    rank = np.arange(n) - starts[wA]
    col = colofs[wA] + rank // 8
    j = rank % 8

    xu3 = np.zeros((TCP * 8, D), BF16)
    xu3[col * 8 + j] = x_bf[u_s[ordA]]
    # [TCP, 8, 16] -> [8, 16, TCP] -> [128, TCP], row = 16j+i
    XU = np.ascontiguousarray(
        xu3.reshape(TCP, 8, D).transpose(1, 2, 0).reshape(128, TCP)
    )
    col_of_edge = np.empty(n, np.int64)
    col_of_edge[ordA] = col
    j_of_edge = np.empty(n, np.int64)
    j_of_edge[ordA] = j
    return XU, col_of_edge, j_of_edge


def kernel(x, W, u, v, widx):
    x = np.asarray(x, np.float32)
    W = np.asarray(W, np.float32)
    u = np.asarray(u).astype(np.int64)
    v = np.asarray(v).astype(np.int64)
    widx = np.asarray(widx).astype(np.int64)

    x_bf = x.astype(BF16)

    # compact j-replicated weight bank: W8[16j+i, 256o+g] = W[g, o, i]
    # (bank-transposed so a group's lhsT is a single-stride AP: the
    # striped SBUF operand bd[16j+i, 256*(16j+o)+g] reads as stride-256)
    WT = W.transpose(2, 1, 0)                          # [i, o, g]
    W8 = np.broadcast_to(WT[None], (8, D, D, NW))
    W8 = np.ascontiguousarray(W8.reshape(128, NW * D)).astype(BF16)

    # ---- shard by destination range -----------------------------------
    shard = v // VSH
    sel = [shard == m for m in range(N_CORES)]
    u_s = [u[s] for s in sel]
    v_s = [v[s] - m * VSH for m, s in enumerate(sel)]
    w_s = [widx[s] for s in sel]

    # ---- common A-side structure (max group size across cores) --------
    cnts = np.stack([np.bincount(ws, minlength=NW) for ws in w_s])   # [8, NW]
    NC = (cnts.max(axis=0) + 7) // 8                                 # cols per group
    NC = np.maximum(NC, 1)
    colofs = np.zeros(NW + 1, np.int64)
    np.cumsum(NC, out=colofs[1:])
    TC = int(colofs[-1])
    TCP = ((TC + CHUNK - 1) // CHUNK) * CHUNK

    units = []
    for g in range(NW):
        c = int(colofs[g])
        b = c + int(NC[g])
        while c < b:
            lim = min(b, (c // PSB + 1) * PSB)
            units.append((g, c, lim))
            c = lim

    # ---- common B-side structure (degree-sorted windows) --------------
    degs = np.stack([np.bincount(vs, minlength=VSH) for vs in v_s])  # [8, VSH]
    perms = [np.argsort(-degs[m], kind="stable") for m in range(N_CORES)]
    sdeg = np.stack([degs[m][perms[m]] for m in range(N_CORES)])     # desc
    DN = sdeg[:, ::128].max(axis=0).astype(np.int64)                 # [NWIN]
    DN = np.maximum(DN, 1)
    woff = np.zeros(NWIN + 1, np.int64)
    np.cumsum(DN * 128 * D, out=woff[1:])
    TOTB = int(woff[-1])

    runs = []
    k = 0
    while k < NWIN:
        k2 = k
        while k2 < NWIN and DN[k2] == DN[k]:
            k2 += 1
        # chunk runs so each sbuf tile stays small
        dn = int(DN[k])
        max_nw = max(1, B_MAX_FREE // (D * dn))
        while k < k2:
            k1 = min(k2, k + max_nw)
            runs.append((dn, k, k1, int(woff[k])))
            k = k1

    # ---- host prep per core -------------------------------------------
    prepsA = [_prep_a(u_s[m], w_s[m], x_bf, colofs, TCP) for m in range(N_CORES)]

    # ---- launch A: per-edge transform ---------------------------------
    ncA = _build_kernel_a(TCP, units)
    in_maps_a = [{"XU": p[0], "W8": W8} for p in prepsA]
    LAST_EXEC_NS.clear()
    resA = run_bass_kernel_spmd(ncA, in_maps_a, list(range(N_CORES)), trace=TRACE)
    if TRACE:
        LAST_EXEC_NS.append(resA.exec_time_ns)

    # ---- host: permute messages widx-order -> v-order -----------------
    in_maps_b = []
    for m in range(N_CORES):
        msgsA = resA.results[m]["MSG"]                # [128, TCP] bf16
        _, col, j = prepsA[m]
        vecs = msgsA[(j * D)[:, None] + np.arange(D)[None, :], col[:, None]]

        vs = v_s[m]
        ordB = np.argsort(vs, kind="stable")
        vB = vs[ordB]
        deg = degs[m]
        startsB = np.zeros(VSH + 1, np.int64)
        np.cumsum(deg, out=startsB[1:])
        s_of = np.arange(vB.shape[0]) - startsB[vB]   # slot within node
        rank_of_node = np.empty(VSH, np.int64)
        rank_of_node[perms[m]] = np.arange(VSH)
        r = rank_of_node[vB]
        kw = r // 128
        p = r % 128
        dnk = DN[kw]
        base = woff[kw] + (p * D) * dnk + s_of
        flat = np.zeros(TOTB, BF16)
        flat[base[:, None] + np.arange(D)[None, :] * dnk[:, None]] = vecs[ordB]
        in_maps_b.append({"MSGB": flat})

    # ---- launch B: segment-sum + ReLU ---------------------------------
    ncB = _build_kernel_b(TOTB, runs)
    resB = run_bass_kernel_spmd(ncB, in_maps_b, list(range(N_CORES)), trace=TRACE)
    if TRACE:
        LAST_EXEC_NS.append(resB.exec_time_ns)

    out = np.empty((N_NODES, D), np.float32)
    for m in range(N_CORES):
        outP = resB.results[m]["OUTP"]                # [128, NWIN*D]
        byrank = outP.reshape(128, NWIN, D).transpose(1, 0, 2).reshape(NWIN * 128, D)
        out[m * VSH + perms[m]] = byrank[:VSH].astype(np.float32)
    return out


# revision 18
# speedup vs baseline: 1.4314x; 1.0881x over previous
"""GNN message-passing (R-GCN style) kernel for 8 Trainium2 NeuronCores.

Reference computation:
    msgs = einsum("eoi,ei->eo", W[widx], x[u])      # per-edge transform
    out  = relu(segment_sum(msgs, v, N))            # scatter-add + relu

Distribution strategy: edges are sharded by destination-node range
(12500 nodes per core), so each core owns a disjoint slice of the output
and no inter-core collective is needed.  W and x are replicated.

Device-side work (all FLOPs):
  Launch A: per-edge weight transform.  Edges are grouped by weight index
    (widx); each group's edges are packed 8-per-column and processed with
    a block-diagonal [128,128] @ [128,ncols] PE matmul (K = 8x16).  The
    block-diagonal weight operand lives in SBUF striped by j-slot
    (stripe j holds the weight bank contiguously at segment j), so it is
    built from the compact 1MB j-replicated bank W8 with 8 fully
    contiguous DMAs plus a zero memset split across three engines; the
    matmul lhsT reads it with a strided [128, 8, 16] access pattern.
    Group column ranges are variable (sized to actual per-group counts,
    maxed across cores so one SPMD program serves all 8 cores).  PSUM is
    accumulated in [128, 512] blocks drained by single large copies
    alternating between the vector and scalar engines.
  Launch B: segment-sum + ReLU.  Destination nodes are bucketed into
    128-node windows by descending degree, so each window is padded only
    to its own max degree (DN_k); windows with equal DN_k are batched
    into single vector tensor_reduce instructions (bf16 in/out engages
    the DVE 2x mode; sums of <=44 bf16 values keep absmax error well
    inside the 2e-2 budget).  ReLU on the scalar engine.

The host does data layout only: sharding, sorting/padding into the
static group structure, gathering x rows into the packed matmul operand,
and permuting the 16-float messages from widx-order to v-order between
the two launches.  (Device-side per-edge random access is not available:
the loadable GPSIMD ucode libraries are absent and indirect DMA has
32B/row descriptor granularity, far too slow for 200K rows/core.)
"""

import sys

sys.path.insert(0, "/opt/trn_rl_repo")

import numpy as np
import ml_dtypes

try:
    # bass_utils imports antenv.axon_hooks when tracing is requested via
    # env; some images lack that module — register a graceful stub so a
    # BASS_TRACE=1 environment degrades to "no trace" instead of crashing.
    import antenv.axon_hooks  # noqa: F401
except ImportError:
    import types

    import antenv

    _hooks = types.ModuleType("antenv.axon_hooks")
    _hooks._hook = None
    _hooks.set_axon_ntff_profile_hook = lambda h: setattr(_hooks, "_hook", h)
    _hooks.get_axon_ntff_profile_hook = lambda: _hooks._hook
    sys.modules["antenv.axon_hooks"] = _hooks
    antenv.axon_hooks = _hooks

import concourse.bacc as bacc
import concourse.mybir as mybir
import concourse.tile as tile
from concourse.bass_utils import run_bass_kernel_spmd

BF16 = ml_dtypes.bfloat16

# set by test harnesses: when True, launches run with trace=True and
# per-launch exec times land in LAST_EXEC_NS
TRACE = False
LAST_EXEC_NS = []

N_NODES = 100000
D = 16
NW = 256
N_CORES = 8
VSH = N_NODES // N_CORES          # 12500 destination nodes per core

CHUNK = 2048                      # A-side columns per DMA chunk
PSB = 512                         # A-side columns per PSUM block
NWIN = (VSH + 127) // 128         # 98 destination 128-node windows per core
B_MAX_FREE = 4096                 # B-side max elems/partition per sbuf tile
B_OUT_BF16 = True                 # B-side reduce/relu/output in bf16 (DVE 2x)


def _build_kernel_a(TCP, units):
    """units = list of (g, c0, c1) absolute column ranges, ascending, each
    within a single PSB-aligned block (and hence a single CHUNK)."""
    nc = bacc.Bacc(None, target_bir_lowering=False, debug=False)
    XU = nc.dram_tensor("XU", [128, TCP], mybir.dt.bfloat16, kind="ExternalInput")
    W8 = nc.dram_tensor("W8", [128, NW * D], mybir.dt.bfloat16, kind="ExternalInput")
    MSG = nc.dram_tensor("MSG", [128, TCP], mybir.dt.bfloat16, kind="ExternalOutput")

    # group units by psum block
    blocks = {}
    for g, c0, c1 in units:
        blocks.setdefault(c0 // PSB, []).append((g, c0, c1))

    with tile.TileContext(nc) as tc:
        with (
            tc.tile_pool(name="bd", bufs=1) as bdp,
            tc.tile_pool(name="sbuf", bufs=3) as pool,
            tc.tile_pool(name="psum", bufs=2, space="PSUM") as psum_pool,
        ):
            # ---- build striped block-diagonal weight operand in SBUF ---
            # stripe j: partitions [16j,16j+16) hold the full bank at
            # segment j (cols [4096j, 4096j+4096)), zeros elsewhere.
            bd = bdp.tile([128, 8 * NW * D], mybir.dt.bfloat16, tag="bd")
            # memset split across engines (4096-aligned so each stripe
            # build-DMA depends on exactly one memset)
            nc.vector.memset(bd[:, 0:12288], 0)
            nc.scalar.memzero(bd[:, 12288:24576])
            nc.gpsimd.memset(bd[:, 24576:8 * NW * D], 0)
            # stripe builds on the gpsimd queue, own-region stripes first
            for j in (6, 7, 0, 1, 2, 3, 4, 5):
                nc.gpsimd.dma_start(
                    out=bd[16 * j:16 * (j + 1), 4096 * j:4096 * (j + 1)],
                    in_=W8[16 * j:16 * (j + 1), :],
                )
            bdv = bd[:].rearrange("p (m g) -> p m g", g=NW)

            nchunks = TCP // CHUNK
            ncopy = 0
            for ch in range(nchunks):
                base = ch * CHUNK
                xu_t = pool.tile([128, CHUNK], mybir.dt.bfloat16, tag="xu")
                # sync queue carries only XU loads: streams from t=0
                nc.sync.dma_start(out=xu_t[:], in_=XU[:, base:base + CHUNK])
                out_t = pool.tile([128, CHUNK], mybir.dt.bfloat16, tag="out")
                for b in range(base // PSB, (base + CHUNK) // PSB):
                    us = blocks.get(b, [])
                    if not us:
                        continue
                    ps = psum_pool.tile([128, PSB], mybir.dt.float32, tag=f"ps{b % 4}")
                    for g, c0, c1 in us:
                        nc.tensor.matmul(
                            out=ps[:, c0 - b * PSB:c1 - b * PSB],
                            lhsT=bdv[:, :, g:g + 1],
                            rhs=xu_t[:, c0 - base:c1 - base],
                            start=True,
                            stop=True,
                        )
                    lo_ps = us[0][1] - b * PSB
                    hi_ps = us[-1][2] - b * PSB
                    lo = us[0][1] - base
                    hi = us[-1][2] - base
                    if ncopy % 2 == 0:
                        nc.vector.tensor_copy(out_t[:, lo:hi], ps[:, lo_ps:hi_ps])
                    else:
                        nc.scalar.copy(out=out_t[:, lo:hi], in_=ps[:, lo_ps:hi_ps])
                    ncopy += 1
                # half-chunk stores on the scalar/gpsimd queues; subtile
                # deps let the first half ship while the second computes
                nc.scalar.dma_start(
                    out=MSG[:, base:base + CHUNK // 2],
                    in_=out_t[:, :CHUNK // 2],
                )
                nc.gpsimd.dma_start(
                    out=MSG[:, base + CHUNK // 2:base + CHUNK],
                    in_=out_t[:, CHUNK // 2:],
                )
    nc.compile()
    return nc


def _build_kernel_b(PT, runs):
    """runs = list of (dn, k0, k1, poff) equal-DN window runs (chunked).
    MSGB is partition-major: MSGB[p, poff_k + o*dn_k + s] = slot s of
    component o of the node at rank k*128+p, so every window-run DMA is a
    fully contiguous 2D slice."""
    nc = bacc.Bacc(None, target_bir_lowering=False, debug=False)
    odt = mybir.dt.bfloat16 if B_OUT_BF16 else mybir.dt.float32
    MSGB = nc.dram_tensor("MSGB", [128, PT], mybir.dt.bfloat16, kind="ExternalInput")
    # partition-major output: OUTP[p, k*D+o] = out of node at rank k*128+p
    OUTP = nc.dram_tensor("OUTP", [128, NWIN * D], odt, kind="ExternalOutput")

    with nc.allow_low_precision("node sums of <=44 bf16 terms fit error budget"), \
            tile.TileContext(nc) as tc:
        with tc.tile_pool(name="sbuf", bufs=6) as pool:
            queues = [nc.sync, nc.gpsimd]
            for ri, (dn, k0, k1, poff) in enumerate(runs):
                nw = k1 - k0
                msg_t = pool.tile([128, nw * D * dn], mybir.dt.bfloat16, tag="msg")
                queues[ri % 2].dma_start(
                    out=msg_t[:],
                    in_=MSGB[:, poff:poff + nw * D * dn],
                )
                acc_t = pool.tile([128, nw * D], odt, tag="acc")
                nc.vector.tensor_reduce(
                    out=acc_t[:],
                    in_=msg_t[:].rearrange("p (w o s) -> p w o s", w=nw, o=D),
                    axis=mybir.AxisListType.X,
                    op=mybir.AluOpType.add,
                )
                out_t = pool.tile([128, nw * D], odt, tag="out")
                nc.scalar.activation(out_t[:], acc_t[:], mybir.ActivationFunctionType.Relu)
                nc.scalar.dma_start(
                    out=OUTP[:, k0 * D:k1 * D],
                    in_=out_t[:],
                )
    nc.compile()
    return nc


def _prep_a(u_s, widx_s, x_bf, colofs, TCP):
    """Pack one core's gathered x rows into the A-side matmul operand.

    Returns (XU [128, TCP] bf16, col(edge), j(edge)) where edge order is
    the stable widx sort of this core's edges.
    """
    ordA = np.argsort(widx_s, kind="stable")
    wA = widx_s[ordA]
    n = u_s.shape[0]
    cnts = np.bincount(wA, minlength=NW)
    starts = np.zeros(NW + 1, np.int64)
    np.cumsum(cnts, out=starts[1:])
    rank = np.arange(n) - starts[wA]
    col = colofs[wA] + rank // 8
    j = rank % 8

    xu3 = np.zeros((TCP * 8, D), BF16)
    xu3[col * 8 + j] = x_bf[u_s[ordA]]
    # [TCP, 8, 16] -> [8, 16, TCP] -> [128, TCP], row = 16j+i
    XU = np.ascontiguousarray(
        xu3.reshape(TCP, 8, D).transpose(1, 2, 0).reshape(128, TCP)
    )
    col_of_edge = np.empty(n, np.int64)
    col_of_edge[ordA] = col
    j_of_edge = np.empty(n, np.int64)
    j_of_edge[ordA] = j
    return XU, col_of_edge, j_of_edge


def kernel(x, W, u, v, widx):
    x = np.asarray(x, np.float32)
    W = np.asarray(W, np.float32)
    u = np.asarray(u).astype(np.int64)
    v = np.asarray(v).astype(np.int64)
    widx = np.asarray(widx).astype(np.int64)

    x_bf = x.astype(BF16)

    # compact j-replicated weight bank: W8[16j+i, 256o+g] = W[g, o, i]
    # (bank-transposed so a group's lhsT is a single-stride AP: the
    # striped SBUF operand bd[16j+i, 256*(16j+o)+g] reads as stride-256)
    WT = W.transpose(2, 1, 0)                          # [i, o, g]
    W8 = np.broadcast_to(WT[None], (8, D, D, NW))
    W8 = np.ascontiguousarray(W8.reshape(128, NW * D)).astype(BF16)

    # ---- shard by destination range -----------------------------------
    shard = v // VSH
    sel = [shard == m for m in range(N_CORES)]
    u_s = [u[s] for s in sel]
    v_s = [v[s] - m * VSH for m, s in enumerate(sel)]
    w_s = [widx[s] for s in sel]

    # ---- common A-side structure (max group size across cores) --------
    cnts = np.stack([np.bincount(ws, minlength=NW) for ws in w_s])   # [8, NW]
    NC = (cnts.max(axis=0) + 7) // 8                                 # cols per group
    NC = np.maximum(NC, 1)
    colofs = np.zeros(NW + 1, np.int64)
    np.cumsum(NC, out=colofs[1:])
    TC = int(colofs[-1])
    TCP = ((TC + CHUNK - 1) // CHUNK) * CHUNK

    units = []
    for g in range(NW):
        c = int(colofs[g])
        b = c + int(NC[g])
        while c < b:
            lim = min(b, (c // PSB + 1) * PSB)
            units.append((g, c, lim))
            c = lim

    # ---- common B-side structure (degree-sorted windows) --------------
    degs = np.stack([np.bincount(vs, minlength=VSH) for vs in v_s])  # [8, VSH]
    perms = [np.argsort(-degs[m], kind="stable") for m in range(N_CORES)]
    sdeg = np.stack([degs[m][perms[m]] for m in range(N_CORES)])     # desc
    DN = sdeg[:, ::128].max(axis=0).astype(np.int64)                 # [NWIN]
    DN = np.maximum(DN, 1)
    poff = np.zeros(NWIN + 1, np.int64)
    np.cumsum(DN * D, out=poff[1:])
    PT = int(poff[-1])

    runs = []
    k = 0
    while k < NWIN:
        k2 = k
        while k2 < NWIN and DN[k2] == DN[k]:
            k2 += 1
        # chunk runs so each sbuf tile stays small
        dn = int(DN[k])
        max_nw = max(1, B_MAX_FREE // (D * dn))
        while k < k2:
            k1 = min(k2, k + max_nw)
            runs.append((dn, k, k1, int(poff[k])))
            k = k1

    # ---- host prep per core -------------------------------------------
    prepsA = [_prep_a(u_s[m], w_s[m], x_bf, colofs, TCP) for m in range(N_CORES)]

    # ---- launch A: per-edge transform ---------------------------------
    ncA = _build_kernel_a(TCP, units)
    in_maps_a = [{"XU": p[0], "W8": W8} for p in prepsA]
    LAST_EXEC_NS.clear()
    resA = run_bass_kernel_spmd(ncA, in_maps_a, list(range(N_CORES)), trace=TRACE)
    if TRACE:
        LAST_EXEC_NS.append(resA.exec_time_ns)

    # ---- host: permute messages widx-order -> v-order -----------------
    in_maps_b = []
    for m in range(N_CORES):
        msgsA = resA.results[m]["MSG"]                # [128, TCP] bf16
        _, col, j = prepsA[m]
        vecs = msgsA[(j * D)[:, None] + np.arange(D)[None, :], col[:, None]]

        vs = v_s[m]
        ordB = np.argsort(vs, kind="stable")
        vB = vs[ordB]
        deg = degs[m]
        startsB = np.zeros(VSH + 1, np.int64)
        np.cumsum(deg, out=startsB[1:])
        s_of = np.arange(vB.shape[0]) - startsB[vB]   # slot within node
        rank_of_node = np.empty(VSH, np.int64)
        rank_of_node[perms[m]] = np.arange(VSH)
        r = rank_of_node[vB]
        kw = r // 128
        p = r % 128
        dnk = DN[kw]
        base = poff[kw] + s_of
        flat = np.zeros((128, PT), BF16)
        flat[p[:, None], base[:, None] + np.arange(D)[None, :] * dnk[:, None]] = vecs[ordB]
        in_maps_b.append({"MSGB": flat})

    # ---- launch B: segment-sum + ReLU ---------------------------------
    ncB = _build_kernel_b(PT, runs)
    resB = run_bass_kernel_spmd(ncB, in_maps_b, list(range(N_CORES)), trace=TRACE)
    if TRACE:
        LAST_EXEC_NS.append(resB.exec_time_ns)

    out = np.empty((N_NODES, D), np.float32)
    for m in range(N_CORES):
        outP = resB.results[m]["OUTP"]                # [128, NWIN*D]
        byrank = outP.reshape(128, NWIN, D).transpose(1, 0, 2).reshape(NWIN * 128, D)
        out[m * VSH + perms[m]] = byrank[:VSH].astype(np.float32)
    return out


# revision 21
# speedup vs baseline: 1.5241x; 1.0648x over previous
"""GNN message-passing (R-GCN style) kernel for 8 Trainium2 NeuronCores.

Reference computation:
    msgs = einsum("eoi,ei->eo", W[widx], x[u])      # per-edge transform
    out  = relu(segment_sum(msgs, v, N))            # scatter-add + relu

Distribution strategy: edges are sharded by destination-node range
(12500 nodes per core), so each core owns a disjoint slice of the output
and no inter-core collective is needed.  W and x are replicated.

Device-side work (all FLOPs):
  Launch A: per-edge weight transform.  Edges are grouped by weight index
    (widx); each group's edges are packed 8-per-column and processed with
    a block-diagonal [128,128] @ [128,ncols] PE matmul (K = 8x16).  The
    block-diagonal weight operand lives in SBUF striped by j-slot
    (stripe j holds the weight bank contiguously at segment j), so it is
    built from the compact 1MB j-replicated bank W8 with 8 fully
    contiguous DMAs plus a zero memset split across three engines; the
    matmul lhsT reads it with a strided [128, 8, 16] access pattern.
    Group column ranges are variable (sized to actual per-group counts,
    maxed across cores so one SPMD program serves all 8 cores).  PSUM is
    accumulated in [128, 512] blocks drained by single large copies
    alternating between the vector and scalar engines.
  Launch B: segment-sum + ReLU.  Destination nodes are bucketed into
    128-node windows by descending degree, so each window is padded only
    to its own max degree (DN_k); windows with equal DN_k are batched
    into single vector tensor_reduce instructions (bf16 in/out engages
    the DVE 2x mode; sums of <=44 bf16 values keep absmax error well
    inside the 2e-2 budget).  ReLU on the scalar engine.

The host does data layout only: sharding, sorting/padding into the
static group structure, gathering x rows into the packed matmul operand,
and permuting the 16-float messages from widx-order to v-order between
the two launches.  (Device-side per-edge random access is not available:
the loadable GPSIMD ucode libraries are absent and indirect DMA has
32B/row descriptor granularity, far too slow for 200K rows/core.)
"""

import sys

sys.path.insert(0, "/opt/trn_rl_repo")

import numpy as np
import ml_dtypes

try:
    # bass_utils imports antenv.axon_hooks when tracing is requested via
    # env; some images lack that module — register a graceful stub so a
    # BASS_TRACE=1 environment degrades to "no trace" instead of crashing.
    import antenv.axon_hooks  # noqa: F401
except ImportError:
    import types

    import antenv

    _hooks = types.ModuleType("antenv.axon_hooks")
    _hooks._hook = None
    _hooks.set_axon_ntff_profile_hook = lambda h: setattr(_hooks, "_hook", h)
    _hooks.get_axon_ntff_profile_hook = lambda: _hooks._hook
    sys.modules["antenv.axon_hooks"] = _hooks
    antenv.axon_hooks = _hooks

import concourse.bacc as bacc
import concourse.mybir as mybir
import concourse.tile as tile
from concourse.bass_utils import run_bass_kernel_spmd

BF16 = ml_dtypes.bfloat16

# set by test harnesses: when True, launches run with trace=True and
# per-launch exec times land in LAST_EXEC_NS
TRACE = False
LAST_EXEC_NS = []

N_NODES = 100000
D = 16
NW = 256
N_CORES = 8
VSH = N_NODES // N_CORES          # 12500 destination nodes per core

CHUNK = 2048                      # A-side columns per DMA chunk
PSB = 512                         # A-side columns per PSUM block
NWIN = (VSH + 127) // 128         # 98 destination 128-node windows per core
B_MAX_FREE = 4096                 # B-side max elems/partition per sbuf tile
B_OUT_BF16 = True                 # B-side reduce/relu/output in bf16 (DVE 2x)


def _build_kernel_a(TCP, units):
    """units = list of (g, c0, c1) absolute column ranges, ascending, each
    within a single PSB-aligned block (and hence a single CHUNK)."""
    nc = bacc.Bacc(None, target_bir_lowering=False, debug=False)
    XU = nc.dram_tensor("XU", [128, TCP], mybir.dt.bfloat16, kind="ExternalInput")
    W8 = nc.dram_tensor("W8", [128, NW * D], mybir.dt.bfloat16, kind="ExternalInput")
    MSG = nc.dram_tensor("MSG", [128, TCP], mybir.dt.bfloat16, kind="ExternalOutput")

    # group units by psum block
    blocks = {}
    for g, c0, c1 in units:
        blocks.setdefault(c0 // PSB, []).append((g, c0, c1))

    with tile.TileContext(nc) as tc:
        with (
            tc.tile_pool(name="bd", bufs=1) as bdp,
            tc.tile_pool(name="xu", bufs=6) as xu_pool,
            tc.tile_pool(name="sbuf", bufs=3) as pool,
            tc.tile_pool(name="psum", bufs=2, space="PSUM") as psum_pool,
        ):
            # ---- build striped block-diagonal weight operand in SBUF ---
            # stripe j: partitions [16j,16j+16) hold the full bank at
            # segment j (cols [4096j, 4096j+4096)), zeros elsewhere.
            bd = bdp.tile([128, 8 * NW * D], mybir.dt.bfloat16, tag="bd")
            # memset split across engines (4096-aligned so each stripe
            # build-DMA depends on exactly one memset)
            nc.vector.memset(bd[:, 0:12288], 0)
            nc.scalar.memzero(bd[:, 12288:24576])
            nc.gpsimd.memset(bd[:, 24576:8 * NW * D], 0)
            # stripe builds first on the fast sync queue (HWDGE): they
            # gate every matmul, so they go ahead of the XU stream
            for j in (6, 7, 0, 1, 2, 3, 4, 5):
                nc.sync.dma_start(
                    out=bd[16 * j:16 * (j + 1), 4096 * j:4096 * (j + 1)],
                    in_=W8[16 * j:16 * (j + 1), :],
                )
            bdv = bd[:].rearrange("p (m g) -> p m g", g=NW)

            nchunks = TCP // CHUNK
            ncopy = 0
            for ch in range(nchunks):
                base = ch * CHUNK
                xu_t = xu_pool.tile([128, CHUNK], mybir.dt.bfloat16, tag="xu")
                nc.sync.dma_start(out=xu_t[:], in_=XU[:, base:base + CHUNK])
                out_t = pool.tile([128, CHUNK], mybir.dt.bfloat16, tag="out")
                for b in range(base // PSB, (base + CHUNK) // PSB):
                    us = blocks.get(b, [])
                    if not us:
                        continue
                    ps = psum_pool.tile([128, PSB], mybir.dt.float32, tag=f"ps{b % 4}")
                    for g, c0, c1 in us:
                        nc.tensor.matmul(
                            out=ps[:, c0 - b * PSB:c1 - b * PSB],
                            lhsT=bdv[:, :, g:g + 1],
                            rhs=xu_t[:, c0 - base:c1 - base],
                            start=True,
                            stop=True,
                        )
                    lo_ps = us[0][1] - b * PSB
                    hi_ps = us[-1][2] - b * PSB
                    lo = us[0][1] - base
                    hi = us[-1][2] - base
                    if ncopy % 2 == 0:
                        nc.vector.tensor_copy(out_t[:, lo:hi], ps[:, lo_ps:hi_ps])
                    else:
                        nc.scalar.copy(out=out_t[:, lo:hi], in_=ps[:, lo_ps:hi_ps])
                    ncopy += 1
                # half-chunk stores on the scalar/gpsimd queues; subtile
                # deps let the first half ship while the second computes
                nc.scalar.dma_start(
                    out=MSG[:, base:base + CHUNK // 2],
                    in_=out_t[:, :CHUNK // 2],
                )
                nc.gpsimd.dma_start(
                    out=MSG[:, base + CHUNK // 2:base + CHUNK],
                    in_=out_t[:, CHUNK // 2:],
                )
    nc.compile()
    return nc


def _build_kernel_b(PT, runs):
    """runs = list of (dn, k0, k1, poff) equal-DN window runs (chunked).
    MSGB is partition-major: MSGB[p, poff_k + o*dn_k + s] = slot s of
    component o of the node at rank k*128+p, so every window-run DMA is a
    fully contiguous 2D slice."""
    nc = bacc.Bacc(None, target_bir_lowering=False, debug=False)
    odt = mybir.dt.bfloat16 if B_OUT_BF16 else mybir.dt.float32
    MSGB = nc.dram_tensor("MSGB", [128, PT], mybir.dt.bfloat16, kind="ExternalInput")
    # partition-major output: OUTP[p, k*D+o] = out of node at rank k*128+p
    OUTP = nc.dram_tensor("OUTP", [128, NWIN * D], odt, kind="ExternalOutput")

    with nc.allow_low_precision("node sums of <=44 bf16 terms fit error budget"), \
            tile.TileContext(nc) as tc:
        with (
            tc.tile_pool(name="out", bufs=1) as outp,
            tc.tile_pool(name="sbuf", bufs=6) as pool,
        ):
            # single whole-launch output tile -> one contiguous final DMA
            out_t = outp.tile([128, NWIN * D], odt, tag="out")
            queues = [nc.sync, nc.gpsimd, nc.scalar]
            for ri, (dn, k0, k1, poff) in enumerate(runs):
                nw = k1 - k0
                msg_t = pool.tile([128, nw * D * dn], mybir.dt.bfloat16, tag="msg")
                queues[ri % 3].dma_start(
                    out=msg_t[:],
                    in_=MSGB[:, poff:poff + nw * D * dn],
                )
                # two pairwise-add halving levels (dn % 4 == 0) run in the
                # DVE 4x mode, then a short X-reduce of the quarters
                q = dn // 4
                h1 = pool.tile([128, nw * D * (dn // 2)], mybir.dt.bfloat16, tag="h1")
                v = msg_t[:].rearrange("p (w o s) -> p w o s", w=nw, o=D)
                nc.vector.tensor_tensor(
                    out=h1[:], in0=v[:, :, :, :dn // 2], in1=v[:, :, :, dn // 2:],
                    op=mybir.AluOpType.add)
                h2 = pool.tile([128, nw * D * q], mybir.dt.bfloat16, tag="h2")
                v1 = h1[:].rearrange("p (w o s) -> p w o s", w=nw, o=D)
                nc.vector.tensor_tensor(
                    out=h2[:], in0=v1[:, :, :, :q], in1=v1[:, :, :, q:],
                    op=mybir.AluOpType.add)
                acc_t = pool.tile([128, nw * D], odt, tag="acc")
                nc.vector.tensor_reduce(
                    out=acc_t[:],
                    in_=h2[:].rearrange("p (w o s) -> p w o s", w=nw, o=D),
                    axis=mybir.AxisListType.X,
                    op=mybir.AluOpType.add,
                )
                nc.scalar.activation(
                    out_t[:, k0 * D:k1 * D], acc_t[:],
                    mybir.ActivationFunctionType.Relu)
            nc.scalar.dma_start(out=OUTP[:, :], in_=out_t[:])
    nc.compile()
    return nc


def _prep_a(u_s, widx_s, x_bf, colofs, TCP):
    """Pack one core's gathered x rows into the A-side matmul operand.

    Returns (XU [128, TCP] bf16, col(edge), j(edge)) where edge order is
    the stable widx sort of this core's edges.
    """
    ordA = np.argsort(widx_s, kind="stable")
    wA = widx_s[ordA]
    n = u_s.shape[0]
    cnts = np.bincount(wA, minlength=NW)
    starts = np.zeros(NW + 1, np.int64)
    np.cumsum(cnts, out=starts[1:])
    rank = np.arange(n) - starts[wA]
    col = colofs[wA] + rank // 8
    j = rank % 8

    xu3 = np.zeros((TCP * 8, D), BF16)
    xu3[col * 8 + j] = x_bf[u_s[ordA]]
    # [TCP, 8, 16] -> [8, 16, TCP] -> [128, TCP], row = 16j+i
    XU = np.ascontiguousarray(
        xu3.reshape(TCP, 8, D).transpose(1, 2, 0).reshape(128, TCP)
    )
    col_of_edge = np.empty(n, np.int64)
    col_of_edge[ordA] = col
    j_of_edge = np.empty(n, np.int64)
    j_of_edge[ordA] = j
    return XU, col_of_edge, j_of_edge


def kernel(x, W, u, v, widx):
    x = np.asarray(x, np.float32)
    W = np.asarray(W, np.float32)
    u = np.asarray(u).astype(np.int64)
    v = np.asarray(v).astype(np.int64)
    widx = np.asarray(widx).astype(np.int64)

    x_bf = x.astype(BF16)

    # compact j-replicated weight bank: W8[16j+i, 256o+g] = W[g, o, i]
    # (bank-transposed so a group's lhsT is a single-stride AP: the
    # striped SBUF operand bd[16j+i, 256*(16j+o)+g] reads as stride-256)
    WT = W.transpose(2, 1, 0)                          # [i, o, g]
    W8 = np.broadcast_to(WT[None], (8, D, D, NW))
    W8 = np.ascontiguousarray(W8.reshape(128, NW * D)).astype(BF16)

    # ---- shard by destination range -----------------------------------
    shard = v // VSH
    sel = [shard == m for m in range(N_CORES)]
    u_s = [u[s] for s in sel]
    v_s = [v[s] - m * VSH for m, s in enumerate(sel)]
    w_s = [widx[s] for s in sel]

    # ---- common A-side structure (max group size across cores) --------
    cnts = np.stack([np.bincount(ws, minlength=NW) for ws in w_s])   # [8, NW]
    NC = (cnts.max(axis=0) + 7) // 8                                 # cols per group
    NC = np.maximum(NC, 1)
    colofs = np.zeros(NW + 1, np.int64)
    np.cumsum(NC, out=colofs[1:])
    TC = int(colofs[-1])
    TCP = ((TC + CHUNK - 1) // CHUNK) * CHUNK

    units = []
    for g in range(NW):
        c = int(colofs[g])
        b = c + int(NC[g])
        while c < b:
            lim = min(b, (c // PSB + 1) * PSB)
            units.append((g, c, lim))
            c = lim

    # ---- common B-side structure (degree-sorted windows) --------------
    degs = np.stack([np.bincount(vs, minlength=VSH) for vs in v_s])  # [8, VSH]
    perms = [np.argsort(-degs[m], kind="stable") for m in range(N_CORES)]
    sdeg = np.stack([degs[m][perms[m]] for m in range(N_CORES)])     # desc
    DN = sdeg[:, ::128].max(axis=0).astype(np.int64)                 # [NWIN]
    DN = (np.maximum(DN, 1) + 3) // 4 * 4      # mult of 4 for add-halving
    poff = np.zeros(NWIN + 1, np.int64)
    np.cumsum(DN * D, out=poff[1:])
    PT = int(poff[-1])

    runs = []
    k = 0
    while k < NWIN:
        k2 = k
        while k2 < NWIN and DN[k2] == DN[k]:
            k2 += 1
        # chunk runs so each sbuf tile stays small
        dn = int(DN[k])
        max_nw = max(1, B_MAX_FREE // (D * dn))
        while k < k2:
            k1 = min(k2, k + max_nw)
            runs.append((dn, k, k1, int(poff[k])))
            k = k1

    # ---- host prep per core -------------------------------------------
    prepsA = [_prep_a(u_s[m], w_s[m], x_bf, colofs, TCP) for m in range(N_CORES)]

    # ---- launch A: per-edge transform ---------------------------------
    ncA = _build_kernel_a(TCP, units)
    in_maps_a = [{"XU": p[0], "W8": W8} for p in prepsA]
    LAST_EXEC_NS.clear()
    resA = run_bass_kernel_spmd(ncA, in_maps_a, list(range(N_CORES)), trace=TRACE)
    if TRACE:
        LAST_EXEC_NS.append(resA.exec_time_ns)

    # ---- host: permute messages widx-order -> v-order -----------------
    in_maps_b = []
    for m in range(N_CORES):
        msgsA = resA.results[m]["MSG"]                # [128, TCP] bf16
        _, col, j = prepsA[m]
        vecs = msgsA[(j * D)[:, None] + np.arange(D)[None, :], col[:, None]]

        vs = v_s[m]
        ordB = np.argsort(vs, kind="stable")
        vB = vs[ordB]
        deg = degs[m]
        startsB = np.zeros(VSH + 1, np.int64)
        np.cumsum(deg, out=startsB[1:])
        s_of = np.arange(vB.shape[0]) - startsB[vB]   # slot within node
        rank_of_node = np.empty(VSH, np.int64)
        rank_of_node[perms[m]] = np.arange(VSH)
        r = rank_of_node[vB]
        kw = r // 128
        p = r % 128
        dnk = DN[kw]
        base = poff[kw] + s_of
        flat = np.zeros((128, PT), BF16)
        flat[p[:, None], base[:, None] + np.arange(D)[None, :] * dnk[:, None]] = vecs[ordB]
        in_maps_b.append({"MSGB": flat})

    # ---- launch B: segment-sum + ReLU ---------------------------------
    ncB = _build_kernel_b(PT, runs)
    resB = run_bass_kernel_spmd(ncB, in_maps_b, list(range(N_CORES)), trace=TRACE)
    if TRACE:
        LAST_EXEC_NS.append(resB.exec_time_ns)

    out = np.empty((N_NODES, D), np.float32)
    for m in range(N_CORES):
        outP = resB.results[m]["OUTP"]                # [128, NWIN*D]
        byrank = outP.reshape(128, NWIN, D).transpose(1, 0, 2).reshape(NWIN * 128, D)
        out[m * VSH + perms[m]] = byrank[:VSH].astype(np.float32)
    return out


# revision 25
# speedup vs baseline: 1.5717x; 1.0312x over previous
"""GNN message-passing (R-GCN style) kernel for 8 Trainium2 NeuronCores.

Reference computation:
    msgs = einsum("eoi,ei->eo", W[widx], x[u])      # per-edge transform
    out  = relu(segment_sum(msgs, v, N))            # scatter-add + relu

Distribution strategy: edges are sharded by destination-node range
(12500 nodes per core), so each core owns a disjoint slice of the output
and no inter-core collective is needed.  W and x are replicated.

Device-side work (all FLOPs):
  Launch A: per-edge weight transform.  Edges are grouped by weight index
    (widx); each group's edges are packed 8-per-column and processed with
    a block-diagonal [128,128] @ [128,ncols] PE matmul (K = 8x16).  The
    block-diagonal weight operand lives in SBUF striped by j-slot
    (stripe j holds the weight bank contiguously at segment j), so it is
    built from the compact 1MB j-replicated bank W8 with 8 fully
    contiguous DMAs plus a zero memset split across three engines; the
    matmul lhsT reads it with a strided [128, 8, 16] access pattern.
    Group column ranges are variable (sized to actual per-group counts,
    maxed across cores so one SPMD program serves all 8 cores).  PSUM is
    accumulated in [128, 512] blocks drained by single large copies
    alternating between the vector and scalar engines.
  Launch B: segment-sum + ReLU.  Destination nodes are bucketed into
    128-node windows by descending degree, so each window is padded only
    to its own max degree (DN_k); windows with equal DN_k are batched
    into single vector tensor_reduce instructions (bf16 in/out engages
    the DVE 2x mode; sums of <=44 bf16 values keep absmax error well
    inside the 2e-2 budget).  ReLU on the scalar engine.

The host does data layout only: sharding, sorting/padding into the
static group structure, gathering x rows into the packed matmul operand,
and permuting the 16-float messages from widx-order to v-order between
the two launches.  (Device-side per-edge random access is not available:
the loadable GPSIMD ucode libraries are absent and indirect DMA has
32B/row descriptor granularity, far too slow for 200K rows/core.)
"""

import sys

sys.path.insert(0, "/opt/trn_rl_repo")

import numpy as np
import ml_dtypes

try:
    # bass_utils imports antenv.axon_hooks when tracing is requested via
    # env; some images lack that module — register a graceful stub so a
    # BASS_TRACE=1 environment degrades to "no trace" instead of crashing.
    import antenv.axon_hooks  # noqa: F401
except ImportError:
    import types

    import antenv

    _hooks = types.ModuleType("antenv.axon_hooks")
    _hooks._hook = None
    _hooks.set_axon_ntff_profile_hook = lambda h: setattr(_hooks, "_hook", h)
    _hooks.get_axon_ntff_profile_hook = lambda: _hooks._hook
    sys.modules["antenv.axon_hooks"] = _hooks
    antenv.axon_hooks = _hooks

import concourse.bacc as bacc
import concourse.mybir as mybir
import concourse.tile as tile
from concourse.bass_utils import run_bass_kernel_spmd

BF16 = ml_dtypes.bfloat16

# set by test harnesses: when True, launches run with trace=True and
# per-launch exec times land in LAST_EXEC_NS
TRACE = False
LAST_EXEC_NS = []

N_NODES = 100000
D = 16
NW = 256
N_CORES = 8
VSH = N_NODES // N_CORES          # 12500 destination nodes per core

CHUNK = 2048                      # A-side columns per DMA chunk
PSB = 512                         # A-side columns per PSUM block
NWIN = (VSH + 127) // 128         # 98 destination 128-node windows per core
B_MAX_FREE = 4096                 # B-side max elems/partition per sbuf tile
B_OUT_BF16 = True                 # B-side reduce/relu/output in bf16 (DVE 2x)


def _build_kernel_a(TCP, units):
    """units = list of (g, c0, c1) absolute column ranges, ascending, each
    within a single PSB-aligned block (and hence a single CHUNK)."""
    nc = bacc.Bacc(None, target_bir_lowering=False, debug=False)
    XU = nc.dram_tensor("XU", [128, TCP], mybir.dt.bfloat16, kind="ExternalInput")
    W8 = nc.dram_tensor("W8", [128, NW * D], mybir.dt.bfloat16, kind="ExternalInput")
    MSG = nc.dram_tensor("MSG", [128, TCP], mybir.dt.bfloat16, kind="ExternalOutput")

    # group units by psum block
    blocks = {}
    for g, c0, c1 in units:
        blocks.setdefault(c0 // PSB, []).append((g, c0, c1))

    with tile.TileContext(nc) as tc:
        with (
            tc.tile_pool(name="bd", bufs=1) as bdp,
            tc.tile_pool(name="xu", bufs=6) as xu_pool,
            tc.tile_pool(name="sbuf", bufs=3) as pool,
            tc.tile_pool(name="psum", bufs=2, space="PSUM") as psum_pool,
        ):
            # ---- build striped block-diagonal weight operand in SBUF ---
            # stripe j: partitions [16j,16j+16) hold the full bank at
            # segment j (cols [4096j, 4096j+4096)), zeros elsewhere.
            bd = bdp.tile([128, 8 * NW * D], mybir.dt.bfloat16, tag="bd")
            # memset split across engines (4096-aligned so each stripe
            # build-DMA depends on exactly one memset); uint32 views
            # halve the element count each engine has to write
            nc.vector.memset(bd[:, 0:4096].bitcast(mybir.dt.uint32), 0)
            nc.scalar.memzero(bd[:, 4096:16384])
            nc.gpsimd.memset(bd[:, 16384:8 * NW * D].bitcast(mybir.dt.uint32), 0)
            # stripe builds first on the fast sync queue (HWDGE): they
            # gate every matmul, so they go ahead of the XU stream;
            # ordered to chase the engines' expected memset finish times
            for j in (0, 4, 5, 6, 7, 1, 2, 3):
                nc.sync.dma_start(
                    out=bd[16 * j:16 * (j + 1), 4096 * j:4096 * (j + 1)],
                    in_=W8[16 * j:16 * (j + 1), :],
                )
            bdv = bd[:].rearrange("p (m g) -> p m g", g=NW)

            nchunks = TCP // CHUNK
            ncopy = 0
            for ch in range(nchunks):
                base = ch * CHUNK
                xu_t = xu_pool.tile([128, CHUNK], mybir.dt.bfloat16, tag="xu")
                nc.sync.dma_start(out=xu_t[:], in_=XU[:, base:base + CHUNK])
                out_t = pool.tile([128, CHUNK], mybir.dt.bfloat16, tag="out")
                for b in range(base // PSB, (base + CHUNK) // PSB):
                    us = blocks.get(b, [])
                    if not us:
                        continue
                    ps = psum_pool.tile([128, PSB], mybir.dt.float32, tag=f"ps{b % 4}")
                    for g, c0, c1 in us:
                        nc.tensor.matmul(
                            out=ps[:, c0 - b * PSB:c1 - b * PSB],
                            lhsT=bdv[:, :, g:g + 1],
                            rhs=xu_t[:, c0 - base:c1 - base],
                            start=True,
                            stop=True,
                        )
                    lo_ps = us[0][1] - b * PSB
                    hi_ps = us[-1][2] - b * PSB
                    lo = us[0][1] - base
                    hi = us[-1][2] - base
                    if ncopy % 2 == 0:
                        nc.vector.tensor_copy(out_t[:, lo:hi], ps[:, lo_ps:hi_ps])
                    else:
                        nc.scalar.copy(out=out_t[:, lo:hi], in_=ps[:, lo_ps:hi_ps])
                    ncopy += 1
                # half-chunk stores on the gpsimd queue (keeps the scalar
                # engine free for drains); subtile deps let the first
                # half ship while the second computes
                nc.gpsimd.dma_start(
                    out=MSG[:, base:base + CHUNK // 2],
                    in_=out_t[:, :CHUNK // 2],
                )
                nc.gpsimd.dma_start(
                    out=MSG[:, base + CHUNK // 2:base + CHUNK],
                    in_=out_t[:, CHUNK // 2:],
                )
    nc.compile()
    return nc


def _build_kernel_b(PT, runs):
    """runs = list of (dn, k0, k1, poff) equal-DN window runs (chunked).
    MSGB is partition-major: MSGB[p, poff_k + o*dn_k + s] = slot s of
    component o of the node at rank k*128+p, so every window-run DMA is a
    fully contiguous 2D slice."""
    nc = bacc.Bacc(None, target_bir_lowering=False, debug=False)
    odt = mybir.dt.bfloat16 if B_OUT_BF16 else mybir.dt.float32
    MSGB = nc.dram_tensor("MSGB", [128, PT], mybir.dt.bfloat16, kind="ExternalInput")
    # partition-major output: OUTP[p, k*D+o] = out of node at rank k*128+p
    OUTP = nc.dram_tensor("OUTP", [128, NWIN * D], odt, kind="ExternalOutput")

    with nc.allow_low_precision("node sums of <=44 bf16 terms fit error budget"), \
            tile.TileContext(nc) as tc:
        with (
            tc.tile_pool(name="out", bufs=1) as outp,
            tc.tile_pool(name="sbuf", bufs=6) as pool,
        ):
            # single whole-launch output tile -> one contiguous final DMA
            out_t = outp.tile([128, NWIN * D], odt, tag="out")
            queues = [nc.sync, nc.gpsimd, nc.scalar]
            qbytes = [0, 0, 0]
            for ri, (dn, k0, k1, poff) in enumerate(runs):
                nw = k1 - k0
                msg_t = pool.tile([128, nw * D * dn], mybir.dt.bfloat16, tag="msg")
                qi = qbytes.index(min(qbytes))
                qbytes[qi] += nw * D * dn
                queues[qi].dma_start(
                    out=msg_t[:],
                    in_=MSGB[:, poff:poff + nw * D * dn],
                )
                # two pairwise-add halving levels (dn % 4 == 0) run in the
                # DVE 4x mode, then a short X-reduce of the quarters
                q = dn // 4
                h1 = pool.tile([128, nw * D * (dn // 2)], mybir.dt.bfloat16, tag="h1")
                v = msg_t[:].rearrange("p (w o s) -> p w o s", w=nw, o=D)
                nc.vector.tensor_tensor(
                    out=h1[:], in0=v[:, :, :, :dn // 2], in1=v[:, :, :, dn // 2:],
                    op=mybir.AluOpType.add)
                h2 = pool.tile([128, nw * D * q], mybir.dt.bfloat16, tag="h2")
                v1 = h1[:].rearrange("p (w o s) -> p w o s", w=nw, o=D)
                nc.vector.tensor_tensor(
                    out=h2[:], in0=v1[:, :, :, :q], in1=v1[:, :, :, q:],
                    op=mybir.AluOpType.add)
                acc_t = pool.tile([128, nw * D], odt, tag="acc")
                nc.vector.tensor_reduce(
                    out=acc_t[:],
                    in_=h2[:].rearrange("p (w o s) -> p w o s", w=nw, o=D),
                    axis=mybir.AxisListType.X,
                    op=mybir.AluOpType.add,
                )
                nc.scalar.activation(
                    out_t[:, k0 * D:k1 * D], acc_t[:],
                    mybir.ActivationFunctionType.Relu)
            nc.scalar.dma_start(out=OUTP[:, :], in_=out_t[:])
    nc.compile()
    return nc


def _prep_a(u_s, widx_s, x_bf, colofs, TCP):
    """Pack one core's gathered x rows into the A-side matmul operand.

    Returns (XU [128, TCP] bf16, col(edge), j(edge)) where edge order is
    the stable widx sort of this core's edges.
    """
    ordA = np.argsort(widx_s, kind="stable")
    wA = widx_s[ordA]
    n = u_s.shape[0]
    cnts = np.bincount(wA, minlength=NW)
    starts = np.zeros(NW + 1, np.int64)
    np.cumsum(cnts, out=starts[1:])
    rank = np.arange(n) - starts[wA]
    col = colofs[wA] + rank // 8
    j = rank % 8

    xu3 = np.zeros((TCP * 8, D), BF16)
    xu3[col * 8 + j] = x_bf[u_s[ordA]]
    # [TCP, 8, 16] -> [8, 16, TCP] -> [128, TCP], row = 16j+i
    XU = np.ascontiguousarray(
        xu3.reshape(TCP, 8, D).transpose(1, 2, 0).reshape(128, TCP)
    )
    col_of_edge = np.empty(n, np.int64)
    col_of_edge[ordA] = col
    j_of_edge = np.empty(n, np.int64)
    j_of_edge[ordA] = j
    return XU, col_of_edge, j_of_edge


def kernel(x, W, u, v, widx):
    x = np.asarray(x, np.float32)
    W = np.asarray(W, np.float32)
    u = np.asarray(u).astype(np.int64)
    v = np.asarray(v).astype(np.int64)
    widx = np.asarray(widx).astype(np.int64)

    x_bf = x.astype(BF16)

    # compact j-replicated weight bank: W8[16j+i, 256o+g] = W[g, o, i]
    # (bank-transposed so a group's lhsT is a single-stride AP: the
    # striped SBUF operand bd[16j+i, 256*(16j+o)+g] reads as stride-256)
    WT = W.transpose(2, 1, 0)                          # [i, o, g]
    W8 = np.broadcast_to(WT[None], (8, D, D, NW))
    W8 = np.ascontiguousarray(W8.reshape(128, NW * D)).astype(BF16)

    # ---- shard by destination range -----------------------------------
    shard = v // VSH
    sel = [shard == m for m in range(N_CORES)]
    u_s = [u[s] for s in sel]
    v_s = [v[s] - m * VSH for m, s in enumerate(sel)]
    w_s = [widx[s] for s in sel]

    # ---- common A-side structure (max group size across cores) --------
    cnts = np.stack([np.bincount(ws, minlength=NW) for ws in w_s])   # [8, NW]
    NC = (cnts.max(axis=0) + 7) // 8                                 # cols per group
    NC = np.maximum(NC, 1)
    colofs = np.zeros(NW + 1, np.int64)
    np.cumsum(NC, out=colofs[1:])
    TC = int(colofs[-1])
    TCP = ((TC + CHUNK - 1) // CHUNK) * CHUNK

    units = []
    for g in range(NW):
        c = int(colofs[g])
        b = c + int(NC[g])
        while c < b:
            lim = min(b, (c // PSB + 1) * PSB)
            units.append((g, c, lim))
            c = lim

    # ---- common B-side structure (degree-sorted windows) --------------
    degs = np.stack([np.bincount(vs, minlength=VSH) for vs in v_s])  # [8, VSH]
    perms = [np.argsort(-degs[m], kind="stable") for m in range(N_CORES)]
    sdeg = np.stack([degs[m][perms[m]] for m in range(N_CORES)])     # desc
    DN = sdeg[:, ::128].max(axis=0).astype(np.int64)                 # [NWIN]
    DN = (np.maximum(DN, 1) + 3) // 4 * 4      # mult of 4 for add-halving
    poff = np.zeros(NWIN + 1, np.int64)
    np.cumsum(DN * D, out=poff[1:])
    PT = int(poff[-1])

    runs = []
    k = 0
    while k < NWIN:
        k2 = k
        while k2 < NWIN and DN[k2] == DN[k]:
            k2 += 1
        # chunk runs so each sbuf tile stays small
        dn = int(DN[k])
        max_nw = max(1, B_MAX_FREE // (D * dn))
        while k < k2:
            k1 = min(k2, k + max_nw)
            runs.append((dn, k, k1, int(poff[k])))
            k = k1

    # ---- host prep per core -------------------------------------------
    prepsA = [_prep_a(u_s[m], w_s[m], x_bf, colofs, TCP) for m in range(N_CORES)]

    # ---- launch A: per-edge transform ---------------------------------
    ncA = _build_kernel_a(TCP, units)
    in_maps_a = [{"XU": p[0], "W8": W8} for p in prepsA]
    LAST_EXEC_NS.clear()
    resA = run_bass_kernel_spmd(ncA, in_maps_a, list(range(N_CORES)), trace=TRACE)
    if TRACE:
        LAST_EXEC_NS.append(resA.exec_time_ns)

    # ---- host: permute messages widx-order -> v-order -----------------
    in_maps_b = []
    for m in range(N_CORES):
        msgsA = resA.results[m]["MSG"]                # [128, TCP] bf16
        _, col, j = prepsA[m]
        vecs = msgsA[(j * D)[:, None] + np.arange(D)[None, :], col[:, None]]

        vs = v_s[m]
        ordB = np.argsort(vs, kind="stable")
        vB = vs[ordB]
        deg = degs[m]
        startsB = np.zeros(VSH + 1, np.int64)
        np.cumsum(deg, out=startsB[1:])
        s_of = np.arange(vB.shape[0]) - startsB[vB]   # slot within node
        rank_of_node = np.empty(VSH, np.int64)
        rank_of_node[perms[m]] = np.arange(VSH)
        r = rank_of_node[vB]
        kw = r // 128
        p = r % 128
        dnk = DN[kw]
        base = poff[kw] + s_of
        flat = np.zeros((128, PT), BF16)
        flat[p[:, None], base[:, None] + np.arange(D)[None, :] * dnk[:, None]] = vecs[ordB]
        in_maps_b.append({"MSGB": flat})

    # ---- launch B: segment-sum + ReLU ---------------------------------
    ncB = _build_kernel_b(PT, runs)
    resB = run_bass_kernel_spmd(ncB, in_maps_b, list(range(N_CORES)), trace=TRACE)
    if TRACE:
        LAST_EXEC_NS.append(resB.exec_time_ns)

    out = np.empty((N_NODES, D), np.float32)
    for m in range(N_CORES):
        outP = resB.results[m]["OUTP"]                # [128, NWIN*D]
        byrank = outP.reshape(128, NWIN, D).transpose(1, 0, 2).reshape(NWIN * 128, D)
        out[m * VSH + perms[m]] = byrank[:VSH].astype(np.float32)
    return out
